# revision 3
# baseline (speedup 1.0000x reference)
import os
import sys

sys.path.insert(0, "/opt/trn_rl_repo")

import numpy as np
import ml_dtypes

import concourse.bass as bass
import concourse.tile as tile
import concourse.mybir as mybir
from concourse import bacc
from concourse.bass import ts
from concourse.bass_utils import run_bass_kernel_spmd

N_CORES = 8
C = 32
SIZE = 128
N_FULL = 50000

SCALE_P = 63.5  # (size-1)/2
DELTA_P = 0.0625 * 63.5  # sample spacing in pixel units = 3.96875

F32 = mybir.dt.float32
F16 = mybir.dt.float16
I32 = mybir.dt.int32

AluOp = mybir.AluOpType
ActFn = mybir.ActivationFunctionType

# x-pair offsets within the gathered 10-voxel span per class
CLASS_OFFS = [(0, 4, 8), (0, 3, 7), (0, 4, 7), (0, 3, 6)]
CLASS_R = [(4, 8), (3, 7), (4, 7), (3, 6)]

_cache = {}


def _emit_tile(nc, tc, pools, tl_out_row, v3, offs, consts, dumps=None):
    """Emit one 128-vertex tile. v3: [128,3] f32 verts view; offs: x-pair
    offsets (r0, r1, r2) in the 10-voxel span for this class."""
    (cpool, gpool, ipool, spool, xpool, zpool, fpool, dpool, pspool, opool) = pools
    mb_sb, bias_sb, vol, out = consts

    # pixel coords for the 3 samples per axis: p9[:, k*3+axis]
    p9 = spool.tile([128, 9], F32, tag="p9")
    for k in range(3):
        nc.scalar.activation(
            p9[:, k * 3 : (k + 1) * 3],
            v3,
            ActFn.Copy,
            bias=SCALE_P + (k - 1) * DELTA_P,
            scale=SCALE_P,
        )
    # floor + frac, robust to cast rounding mode
    ci = spool.tile([128, 9], I32, tag="ci")
    nc.vector.tensor_copy(ci[:], p9[:])
    cf = spool.tile([128, 9], F32, tag="cf")
    nc.vector.tensor_copy(cf[:], ci[:])
    d9 = spool.tile([128, 9], F32, tag="d9")
    nc.vector.tensor_tensor(d9[:], p9[:], cf[:], AluOp.subtract)
    m9 = spool.tile([128, 9], F32, tag="m9")
    nc.vector.tensor_scalar(m9[:], d9[:], 0.0, None, AluOp.is_lt)
    w9 = spool.tile([128, 9], F32, tag="w9")
    nc.vector.tensor_tensor(w9[:], d9[:], m9[:], AluOp.add)
    i9 = spool.tile([128, 9], F32, tag="i9")
    nc.vector.tensor_tensor(i9[:], cf[:], m9[:], AluOp.subtract)

    # gather run index per (kz, ky): (z0*128 + y0)*128 + x0(0), unit = 128 els
    zcols = i9[:, 2:9:3]  # z0(k)
    ycols = i9[:, 1:9:3]
    rz3 = spool.tile([128, 3], F32, tag="rz3")
    nc.vector.tensor_scalar(rz3[:], zcols, 16384.0, None, AluOp.mult)
    ry3 = spool.tile([128, 3], F32, tag="ry3")
    nc.vector.tensor_scalar(ry3[:], ycols, 128.0, None, AluOp.mult)
    zy9 = spool.tile([128, 9], F32, tag="zy9")
    for kz in range(3):
        nc.scalar.activation(
            zy9[:, kz * 3 : (kz + 1) * 3],
            ry3[:],
            ActFn.Identity,
            bias=rz3[:, kz : kz + 1],
            scale=1.0,
        )
    idxf = spool.tile([128, 9], F32, tag="idxf")
    nc.scalar.activation(
        idxf[:], zy9[:], ActFn.Identity, bias=i9[:, 0:1], scale=1.0
    )
    idxi = ipool.tile([128, 9], I32, tag="idxi")
    nc.vector.tensor_copy(idxi[:], idxf[:])
    if dumps:
        nc.sync.dma_start(dumps["d_idx"][:, :], idxi[:])
        nc.sync.dma_start(dumps["d_w9"][:, :], w9[:])

    # gather: 9 runs of [10 x][2 zl][2 yl][32 c] = 1280 els fp16 per vertex
    G = gpool.tile([128, 9, 1280], F16, tag="G")
    for j in range(9):
        nc.gpsimd.indirect_dma_start(
            out=G[:, j, :],
            out_offset=None,
            in_=vol[:, :],
            in_offset=bass.IndirectOffsetOnAxis(ap=idxi[:, j : j + 1], axis=0),
        )

    if dumps:
        nc.sync.dma_start(dumps["d_G"][:, :], G[:].rearrange("p a b -> p (a b)"))
    # x-lerp: pairs at span offsets offs[kx] -> X [9 runs][3 kx][128 (zl yl c)]
    X = xpool.tile([128, 9 * 3 * 128], F16, tag="X")
    Xv = X[:].rearrange("p (r k e) -> p r k e", r=9, k=3)
    Gv = G[:].rearrange("p r (x e) -> p r x e", x=10)
    for kx in range(3):
        A = Gv[:, :, offs[kx], :]
        B = Gv[:, :, offs[kx] + 1, :]
        dx = dpool.tile([128, 9 * 128], F16, tag="dx")
        nc.vector.tensor_tensor(dx[:], B, A, AluOp.subtract)
        nc.vector.scalar_tensor_tensor(
            Xv[:, :, kx, :], dx[:], w9[:, 3 * kx : 3 * kx + 1], A,
            AluOp.mult, AluOp.add,
        )

    if dumps:
        nc.sync.dma_start(dumps["d_X"][:, :], X[:])
    # z-lerp (fold zl): X [kz][(ky kx)][zl][yl c] -> Z [ky][kz][kx][yl][c]
    Z = zpool.tile([128, 27 * 64], F16, tag="Z")
    Xz = X[:].rearrange("p (kz a zl e) -> p kz a zl e", kz=3, a=9, zl=2)
    Zv = Z[:].rearrange("p (ky kz a) -> p ky kz a", ky=3, kz=3)
    for kz in range(3):
        A = Xz[:, kz, :, 0, :]
        B = Xz[:, kz, :, 1, :]
        dz = dpool.tile([128, 9 * 64], F16, tag="dz")
        nc.vector.tensor_tensor(dz[:], B, A, AluOp.subtract)
        nc.vector.scalar_tensor_tensor(
            Zv[:, :, kz, :], dz[:], w9[:, 3 * kz + 2 : 3 * kz + 3], A,
            AluOp.mult, AluOp.add,
        )

    if dumps:
        nc.sync.dma_start(dumps["d_Z"][:, :], Z[:])
    # y-lerp (fold yl): Z [ky][(kz kx)][yl][c] -> F [ky][kz][kx][c]
    F = fpool.tile([128, 896], F16, tag="F")
    Zy = Z[:].rearrange("p (ky a yl c) -> p ky a yl c", ky=3, a=9, yl=2)
    Fv = F[:, 0:864].rearrange("p (ky a) -> p ky a", ky=3)
    for ky in range(3):
        A = Zy[:, ky, :, 0, :]
        B = Zy[:, ky, :, 1, :]
        dy = dpool.tile([128, 9 * C], F16, tag="dy")
        nc.vector.tensor_tensor(dy[:], B, A, AluOp.subtract)
        nc.vector.scalar_tensor_tensor(
            Fv[:, ky, :], dy[:], w9[:, 3 * ky + 1 : 3 * ky + 2], A,
            AluOp.mult, AluOp.add,
        )
    nc.vector.memset(F[:, 864:896], 0.0)

    if dumps:
        nc.sync.dma_start(dumps["d_F"][:, :], F[:])
    FT = fpool.tile([128, 7, 128], F16, tag="FT")
    nc.sync.dma_start_transpose(FT[:], F[:])

    psum = pspool.tile([128, C], F32, tag="ps")
    for t in range(7):
        nc.tensor.matmul(
            psum[:], FT[:, t, :], mb_sb[:, ts(t, C)], start=(t == 0), stop=(t == 6)
        )
    osb = opool.tile([128, C], F32, tag="osb")
    nc.vector.tensor_tensor(osb[:], psum[:], bias_sb[:], AluOp.add)
    nc.sync.dma_start(out[ts(tl_out_row, 128), :], osb[:])


def _build(tile_counts):
    """tile_counts: tuple of per-class 128-vertex tile counts (ta, tb, tc)."""
    tiles = sum(tile_counts)
    nv = tiles * 128
    nc = bacc.Bacc("TRN2", target_bir_lowering=False, debug=False)

    vol = nc.dram_tensor(
        "vol", [SIZE * SIZE * SIZE, 128], F16, kind="ExternalInput"
    ).ap()  # [z y x] rows of [zl yl c] = 128 els
    verts = nc.dram_tensor("verts", [nv, 3], F32, kind="ExternalInput").ap()
    mbig = nc.dram_tensor("mbig", [128, 7 * C], F16, kind="ExternalInput").ap()
    biasr = nc.dram_tensor("biasr", [128, C], F32, kind="ExternalInput").ap()
    out = nc.dram_tensor("out", [nv, C], F32, kind="ExternalOutput").ap()

    with tile.TileContext(nc) as tc:
        with (
            tc.tile_pool(name="const", bufs=1) as cpool,
            tc.tile_pool(name="gather", bufs=3) as gpool,
            tc.tile_pool(name="idx", bufs=3) as ipool,
            tc.tile_pool(name="small", bufs=3) as spool,
            tc.tile_pool(name="xl", bufs=2) as xpool,
            tc.tile_pool(name="zl", bufs=2) as zpool,
            tc.tile_pool(name="fl", bufs=3) as fpool,
            tc.tile_pool(name="dd", bufs=3) as dpool,
            tc.tile_pool(name="psum", bufs=4, space="PSUM") as pspool,
            tc.tile_pool(name="outp", bufs=3) as opool,
        ):
            pools = (cpool, gpool, ipool, spool, xpool, zpool, fpool, dpool,
                     pspool, opool)
            mb_sb = cpool.tile([128, 7 * C], F16, tag="mb")
            nc.sync.dma_start(mb_sb[:], mbig[:])
            bias_sb = cpool.tile([128, C], F32, tag="bias")
            nc.sync.dma_start(bias_sb[:], biasr[:])
            vall = cpool.tile([128, tiles * 3], F32, tag="vall")
            nc.sync.dma_start(vall[:], verts.rearrange("(t p) a -> p t a", p=128))
            consts = (mb_sb, bias_sb, vol, out)

            tl = 0
            for cls, n_t in enumerate(tile_counts):
                for _ in range(n_t):
                    _emit_tile(nc, tc, pools, tl, vall[:, tl * 3 : (tl + 1) * 3],
                               CLASS_OFFS[cls], consts)
                    tl += 1

    nc.compile()
    return nc


def _get_nc(tile_counts):
    key = tuple(tile_counts)
    if key not in _cache:
        _cache[key] = _build(key)
    return _cache[key]


def _host_prep(voxel_features, vertices, w_d1, b_d1, w_d2, b_d2, w_c1, b_c1, w_c2,
               b_c2, conv_w, conv_b):
    # volume -> [z, y, x, zl, yl, c] fp16 (x4 redundant corner-pair layout)
    v = np.transpose(np.asarray(voxel_features, np.float32)[0], (1, 2, 3, 0))
    v = np.ascontiguousarray(v).astype(np.float16)  # [z, y, x, c]
    vp = np.empty((SIZE + 1, SIZE + 1, SIZE, C), np.float16)
    vp[:SIZE, :SIZE] = v
    vp[SIZE, :SIZE] = v[SIZE - 1]
    vp[:SIZE, SIZE] = vp[:SIZE, SIZE - 1]
    vp[SIZE, SIZE] = vp[SIZE, SIZE - 1]
    vol4 = np.empty((SIZE, SIZE, SIZE, 2, 2, C), np.float16)
    for zl in range(2):
        for yl in range(2):
            vol4[:, :, :, zl, yl, :] = vp[zl : zl + SIZE, yl : yl + SIZE]
    vol4 = vol4.reshape(SIZE * SIZE * SIZE, 128)

    f8 = np.float64
    Wd = np.asarray(w_d2, f8) @ np.asarray(w_d1, f8)
    bd = np.asarray(b_d1, f8) @ np.asarray(w_d2, f8).T + np.asarray(b_d2, f8)
    Wc = np.asarray(w_c2, f8) @ np.asarray(w_c1, f8)
    bc = np.asarray(b_c1, f8) @ np.asarray(w_c2, f8).T + np.asarray(b_c2, f8)
    cw = np.asarray(conv_w, f8)[:, :, 0, :]  # [o, c', k]

    A = np.einsum("ock,cd->odk", cw, Wd)  # [o, c, k]
    M = np.moveaxis(A, 2, 0).copy()  # [k, o, c], ref order k = kx*9 + ky*3 + kz
    M[13] += Wc - A.sum(axis=2)
    bias_tot = cw.sum(axis=2) @ bd + np.asarray(conv_b, f8) + bc

    # Mbig row r = ky*288 + kz*96 + kx*32 + c maps M_{kx*9+ky*3+kz}[o, c]
    Mbig = np.zeros((896, C), np.float64)
    for kx in range(3):
        for ky in range(3):
            for kz in range(3):
                r0 = ky * 288 + kz * 96 + kx * 32
                Mbig[r0 : r0 + 32, :] = M[kx * 9 + ky * 3 + kz].T
    mb_host = np.ascontiguousarray(
        Mbig.reshape(7, 128, C).transpose(1, 0, 2).reshape(128, 7 * C)
    ).astype(np.float16)
    biasrep = np.tile(bias_tot.astype(np.float32)[None, :], (128, 1))
    return vol4, mb_host, biasrep


def _classify(vp):
    """vp: [n, 3] f32 vertices -> class id, replicating the device's f32
    arithmetic exactly (p = fl32(v*63.5) + bias_k, floors in f32)."""
    q = vp[:, 0].astype(np.float32) * np.float32(SCALE_P)
    x0 = np.floor(q + np.float32(SCALE_P - DELTA_P)).astype(np.int64)
    x1 = np.floor(q + np.float32(SCALE_P)).astype(np.int64)
    x2 = np.floor(q + np.float32(SCALE_P + DELTA_P)).astype(np.int64)
    r1 = x1 - x0
    r2 = x2 - x0
    cls = np.full(vp.shape[0], -1, np.int64)
    for i, (a, b) in enumerate(CLASS_R):
        cls[(r1 == a) & (r2 == b)] = i
    assert (cls >= 0).all(), "unexpected x-spacing class"
    return cls


def kernel(**inputs):
    vol4, mb_host, biasrep = _host_prep(**inputs)
    vp = np.asarray(inputs["vertices"], np.float32)[0]
    n = vp.shape[0]

    # shard vertices round-robin-contiguous, then class-sort within each core
    per_core = (n + N_CORES - 1) // N_CORES
    in_maps = []
    perms = []
    counts_ref = None
    for i in range(N_CORES):
        seg = vp[i * per_core : min((i + 1) * per_core, n)]
        cls = _classify(seg)
        order = np.argsort(cls, kind="stable")
        seg_sorted = seg[order]
        cls_sorted = cls[order]
        tile_counts = []
        v_parts = []
        for c in range(len(CLASS_OFFS)):
            part = seg_sorted[cls_sorted == c]
            n_t = (len(part) + 127) // 128
            if len(part) < n_t * 128:
                fill = part[:1] if len(part) else None
                pad = np.repeat(fill, n_t * 128 - len(part), axis=0) if fill is not None else None
                part = np.concatenate([part, pad], axis=0) if pad is not None else part
            tile_counts.append(n_t)
            v_parts.append(part)
        verts_padded = np.concatenate(
            [p for p in v_parts if len(p)], axis=0
        ).astype(np.float32)
        if counts_ref is None:
            counts_ref = tuple(tile_counts)
        else:
            # all cores must share one compiled program: equalize tile counts
            counts_ref = tuple(max(a, b) for a, b in zip(counts_ref, tile_counts))
        in_maps.append({"verts": verts_padded, "tile_counts": tuple(tile_counts),
                        "order": order, "seg_len": len(seg)})
        perms.append(order)

    # pad every core's segments up to the common per-class tile counts
    for i in range(N_CORES):
        m = in_maps[i]
        tc_i = m["tile_counts"]
        v = m["verts"]
        pieces = []
        start = 0
        for c in range(len(CLASS_OFFS)):
            seg_c = v[start : start + tc_i[c] * 128]
            start += tc_i[c] * 128
            need = counts_ref[c] * 128
            if len(seg_c) < need:
                fill = seg_c[:1] if len(seg_c) else v[:1]
                seg_c = np.concatenate(
                    [seg_c, np.repeat(fill, need - len(seg_c), axis=0)], axis=0
                )
            pieces.append(seg_c)
        m["verts"] = np.ascontiguousarray(np.concatenate(pieces, axis=0))

    nc = _get_nc(counts_ref)
    run_maps = [
        {"vol": vol4, "verts": in_maps[i]["verts"], "mbig": mb_host,
         "biasr": biasrep}
        for i in range(N_CORES)
    ]
    res = run_bass_kernel_spmd(
        nc, run_maps, list(range(N_CORES)),
        trace=os.environ.get("KBENCH_TRACE", "") == "1",
    )
    globals()["LAST_RESULTS"] = res

    out = np.empty((n, C), np.float32)
    bounds = np.cumsum([0] + [c * 128 for c in counts_ref])
    for i in range(N_CORES):
        seg_len = in_maps[i]["seg_len"]
        raw = res.results[i]["out"]
        # undo per-class padding: concatenate the valid prefix of each class seg
        tc_i = in_maps[i]["tile_counts"]
        cls_sizes = []
        order = in_maps[i]["order"]
        cls = np.zeros(seg_len, np.int64)
        # recompute class sizes from order/classify
        seg = vp[i * per_core : min((i + 1) * per_core, n)]
        c_of = _classify(seg)[order]
        vals = []
        for c in range(len(CLASS_OFFS)):
            k = int((c_of == c).sum())
            vals.append(raw[bounds[c] : bounds[c] + k])
        sorted_out = np.concatenate(vals, axis=0)
        seg_out = np.empty_like(sorted_out)
        seg_out[order] = sorted_out
        out[i * per_core : i * per_core + seg_len] = seg_out
    return out.reshape(1, n, C).astype(np.float32)



# revision 4
# speedup vs baseline: 1.1017x; 1.1017x over previous
import os
import sys

sys.path.insert(0, "/opt/trn_rl_repo")

import numpy as np
import ml_dtypes

import concourse.bass as bass
import concourse.tile as tile
import concourse.mybir as mybir
from concourse import bacc
from concourse.bass import ts
from concourse.bass_utils import run_bass_kernel_spmd

N_CORES = 8
C = 32
SIZE = 128
N_FULL = 50000

SCALE_P = 63.5  # (size-1)/2
DELTA_P = 0.0625 * 63.5  # sample spacing in pixel units = 3.96875

F32 = mybir.dt.float32
F16 = mybir.dt.float16
I32 = mybir.dt.int32

AluOp = mybir.AluOpType
ActFn = mybir.ActivationFunctionType

# x-pair offsets within the gathered 10-voxel span per class
CLASS_OFFS = [(0, 4, 8), (0, 3, 7), (0, 4, 7), (0, 3, 6)]
CLASS_R = [(4, 8), (3, 7), (4, 7), (3, 6)]

_cache = {}


def _emit_preamble(nc, cpool, verts, T):
    """Compute per-vertex fractional weights (w9a) and gather base indices
    (idxia) for ALL tiles in a handful of wide instructions."""
    vall = cpool.tile([128, T * 3], F32, tag="vall")
    nc.sync.dma_start(vall[:], verts.rearrange("(t p) a -> p t a", p=128))
    vall_v = vall[:].rearrange("p (t a) -> p t a", a=3)

    p9a = cpool.tile([128, T * 9], F32, tag="p9a")
    p9a_v = p9a[:].rearrange("p (t n) -> p t n", n=9)
    for k in range(3):
        nc.scalar.activation(
            p9a_v[:, :, k * 3 : (k + 1) * 3],
            vall_v,
            ActFn.Copy,
            bias=SCALE_P + (k - 1) * DELTA_P,
            scale=SCALE_P,
        )
    # floor + frac, robust to cast rounding mode
    ci = cpool.tile([128, T * 9], I32, tag="ci")
    nc.vector.tensor_copy(ci[:], p9a[:])
    cf = cpool.tile([128, T * 9], F32, tag="cf")
    nc.vector.tensor_copy(cf[:], ci[:])
    d9 = cpool.tile([128, T * 9], F32, tag="d9")
    nc.vector.tensor_tensor(d9[:], p9a[:], cf[:], AluOp.subtract)
    m9 = cpool.tile([128, T * 9], F32, tag="m9")
    nc.vector.tensor_scalar(m9[:], d9[:], 0.0, None, AluOp.is_lt)
    w9a = cpool.tile([128, T * 9], F32, tag="w9a")
    nc.vector.tensor_tensor(w9a[:], d9[:], m9[:], AluOp.add)
    i9a = cpool.tile([128, T * 9], F32, tag="i9a")
    nc.vector.tensor_tensor(i9a[:], cf[:], m9[:], AluOp.subtract)

    i9a_v = i9a[:].rearrange("p (t k a) -> p t k a", k=3, a=3)
    rza = cpool.tile([128, T * 3], F32, tag="rza")
    rza_v = rza[:].rearrange("p (t k) -> p t k", k=3)
    nc.vector.tensor_scalar(rza_v, i9a_v[:, :, :, 2], 16384.0, None, AluOp.mult)
    rya = cpool.tile([128, T * 3], F32, tag="rya")
    rya_v = rya[:].rearrange("p (t k) -> p t k", k=3)
    nc.vector.tensor_scalar(rya_v, i9a_v[:, :, :, 1], 128.0, None, AluOp.mult)

    # zy9a[t, kz, ky] = rza[t, kz] + rya[t, ky]
    zy9a = cpool.tile([128, T * 9], F32, tag="zy9a")
    zy9a_v = zy9a[:].rearrange("p (t kz ky) -> p t kz ky", kz=3, ky=3)
    rza_b = rza[:].rearrange("p (t kz o) -> p t kz o", kz=3, o=1).to_broadcast(
        (128, T, 3, 3)
    )
    rya_b = rya[:].rearrange("p (t o ky) -> p t o ky", o=1, ky=3).to_broadcast(
        (128, T, 3, 3)
    )
    nc.vector.tensor_tensor(zy9a_v, rza_b, rya_b, AluOp.add)

    # idx[t, j] = zy9a[t, j] + x0(t)   (x0 = i9a col 0 of each tile)
    idxfa = cpool.tile([128, T * 9], F32, tag="idxfa")
    idxfa_v = idxfa[:].rearrange("p (t n) -> p t n", n=9)
    x0_b = i9a[:].rearrange("p (t n) -> p t n", n=9)[:, :, 0:1].to_broadcast(
        (128, T, 9)
    )
    nc.vector.tensor_tensor(
        idxfa_v, zy9a[:].rearrange("p (t n) -> p t n", n=9), x0_b, AluOp.add
    )
    idxia = cpool.tile([128, T * 9], I32, tag="idxia")
    nc.vector.tensor_copy(idxia[:], idxfa[:])
    return w9a, idxia


def _emit_tile(nc, pools, tl, cls, w9a, idxia, consts):
    (gpool, dpool, wpool, xpool, zpool, fpool, pspool, opool) = pools
    mb_sb, vol, out = consts
    offs = CLASS_OFFS[cls]

    G = gpool.tile([128, 9, 1280], F16, tag="G")
    for j in range(9):
        nc.gpsimd.indirect_dma_start(
            out=G[:, j, :],
            out_offset=None,
            in_=vol[:, :],
            in_offset=bass.IndirectOffsetOnAxis(
                ap=idxia[:, tl * 9 + j : tl * 9 + j + 1], axis=0
            ),
        )

    Gx = G[:].rearrange("p j (x e) -> p j x e", x=10)
    # x-lerp: dxa [kx, r, e] then X[r, kx, e] = A + wx*dx
    dxa = dpool.tile([128, 3 * 9 * 128], F16, tag="dxa")
    dxa_v = dxa[:].rearrange("p (k r e) -> p k r e", k=3, r=9)
    step = offs[1] - offs[0]
    if offs[2] - offs[1] == step:
        A_t = Gx[:, :, offs[0] : offs[2] + 1 : step, :].rearrange(
            "p j k e -> p k j e"
        )
        B_t = Gx[:, :, offs[0] + 1 : offs[2] + 2 : step, :].rearrange(
            "p j k e -> p k j e"
        )
        nc.vector.tensor_tensor(dxa_v, B_t, A_t, AluOp.subtract)
    else:
        for kx in range(3):
            nc.vector.tensor_tensor(
                dxa_v[:, kx],
                Gx[:, :, offs[kx] + 1, :],
                Gx[:, :, offs[kx], :],
                AluOp.subtract,
            )

    X = xpool.tile([128, 9 * 3 * 128], F16, tag="X")
    X_v = X[:].rearrange("p (r k e) -> p r k e", r=9, k=3)
    for kx in range(3):
        wdx = wpool.tile([128, 9 * 128], F16, tag="wdx")
        wdx_v = wdx[:].rearrange("p (r e) -> p r e", r=9)
        nc.scalar.activation(
            wdx_v,
            dxa_v[:, kx],
            ActFn.Copy,
            bias=0.0,
            scale=w9a[:, tl * 9 + 3 * kx : tl * 9 + 3 * kx + 1],
        )
        nc.vector.tensor_tensor(
            X_v[:, :, kx, :], wdx_v, Gx[:, :, offs[kx], :], AluOp.add
        )

    # z-lerp: fold zl. X = [r=(kz,ky), kx, zl, ylc]
    Xz = X[:].rearrange("p (r k zl e) -> p r k zl e", r=9, k=3, zl=2)
    dza = dpool.tile([128, 27 * 64], F16, tag="dza")
    dza_v = dza[:].rearrange("p (kz ky k e) -> p kz ky k e", kz=3, ky=3, k=3)
    nc.vector.tensor_tensor(dza_v, Xz[:, :, :, 1, :].rearrange(
        "p (kz ky) k e -> p kz ky k e", kz=3
    ), Xz[:, :, :, 0, :].rearrange("p (kz ky) k e -> p kz ky k e", kz=3),
        AluOp.subtract)

    Z = zpool.tile([128, 27 * 64], F16, tag="Z")
    Z_v = Z[:].rearrange("p (ky kz k e) -> p ky kz k e", ky=3, kz=3, k=3)
    Xz4 = X[:].rearrange(
        "p (kz ky k zl e) -> p kz ky k zl e", kz=3, ky=3, k=3, zl=2
    )
    for kz in range(3):
        wdz = wpool.tile([128, 9 * 64], F16, tag="wdz")
        wdz_v = wdz[:].rearrange("p (ky k e) -> p ky k e", ky=3, k=3)
        nc.vector.tensor_scalar(
            wdz_v,
            dza_v[:, kz],
            w9a[:, tl * 9 + 3 * kz + 2 : tl * 9 + 3 * kz + 3],
            None,
            AluOp.mult,
        )
        nc.vector.tensor_tensor(
            Z_v[:, :, kz], wdz_v, Xz4[:, kz, :, :, 0, :], AluOp.add
        )

    # y-lerp: fold yl. Z = [ky, kz, kx, yl, c]
    Zy = Z[:].rearrange("p (ky a yl c) -> p ky a yl c", ky=3, a=9, yl=2)
    dya = dpool.tile([128, 27 * C], F16, tag="dya")
    dya_v = dya[:].rearrange("p (ky a c) -> p ky a c", ky=3, a=9)
    nc.vector.tensor_tensor(dya_v, Zy[:, :, :, 1, :], Zy[:, :, :, 0, :],
                            AluOp.subtract)

    F = fpool.tile([128, 896], F16, tag="F")
    F_v = F[:, 0:864].rearrange("p (ky a c) -> p ky a c", ky=3, a=9)
    for ky in range(3):
        wdy = wpool.tile([128, 9 * C], F16, tag="wdy")
        wdy_v = wdy[:].rearrange("p (a c) -> p a c", a=9)
        nc.vector.tensor_scalar(
            wdy_v,
            dya_v[:, ky],
            w9a[:, tl * 9 + 3 * ky + 1 : tl * 9 + 3 * ky + 2],
            None,
            AluOp.mult,
        )
        nc.vector.tensor_tensor(
            F_v[:, ky], wdy_v, Zy[:, ky, :, 0, :], AluOp.add
        )
    nc.vector.memset(F[:, 864:865], 1.0)
    nc.vector.memset(F[:, 865:896], 0.0)

    FT = fpool.tile([128, 7, 128], F16, tag="FT")
    nc.sync.dma_start_transpose(FT[:], F[:])

    psum = pspool.tile([128, C], F32, tag="ps")
    for t in range(7):
        nc.tensor.matmul(
            psum[:], FT[:, t, :], mb_sb[:, ts(t, C)], start=(t == 0), stop=(t == 6)
        )
    osb = opool.tile([128, C], F32, tag="osb")
    nc.scalar.copy(osb[:], psum[:])
    nc.sync.dma_start(out[ts(tl, 128), :], osb[:])


def _build(tile_counts):
    """tile_counts: per-class 128-vertex tile counts."""
    T = sum(tile_counts)
    nv = T * 128
    nc = bacc.Bacc("TRN2", target_bir_lowering=False, debug=False)

    vol = nc.dram_tensor(
        "vol", [SIZE * SIZE * SIZE, 128], F16, kind="ExternalInput"
    ).ap()  # [z y x] rows of [zl yl c] = 128 els
    verts = nc.dram_tensor("verts", [nv, 3], F32, kind="ExternalInput").ap()
    mbig = nc.dram_tensor("mbig", [128, 7 * C], F16, kind="ExternalInput").ap()
    out = nc.dram_tensor("out", [nv, C], F32, kind="ExternalOutput").ap()

    with tile.TileContext(nc) as tc:
        with (
            tc.tile_pool(name="const", bufs=1) as cpool,
            tc.tile_pool(name="gather", bufs=4) as gpool,
            tc.tile_pool(name="dd", bufs=3) as dpool,
            tc.tile_pool(name="wd", bufs=3) as wpool,
            tc.tile_pool(name="xl", bufs=2) as xpool,
            tc.tile_pool(name="zl", bufs=2) as zpool,
            tc.tile_pool(name="fl", bufs=3) as fpool,
            tc.tile_pool(name="psum", bufs=4, space="PSUM") as pspool,
            tc.tile_pool(name="outp", bufs=3) as opool,
        ):
            mb_sb = cpool.tile([128, 7 * C], F16, tag="mb")
            nc.sync.dma_start(mb_sb[:], mbig[:])
            w9a, idxia = _emit_preamble(nc, cpool, verts, T)
            pools = (gpool, dpool, wpool, xpool, zpool, fpool, pspool, opool)
            consts = (mb_sb, vol, out)

            tl = 0
            for cls, n_t in enumerate(tile_counts):
                for _ in range(n_t):
                    _emit_tile(nc, pools, tl, cls, w9a, idxia, consts)
                    tl += 1

    nc.compile()
    return nc


def _get_nc(tile_counts):
    key = tuple(tile_counts)
    if key not in _cache:
        _cache[key] = _build(key)
    return _cache[key]


def _host_prep(voxel_features, vertices, w_d1, b_d1, w_d2, b_d2, w_c1, b_c1, w_c2,
               b_c2, conv_w, conv_b):
    # volume -> [z, y, x, zl, yl, c] fp16 (x4 redundant corner-pair layout)
    v = np.transpose(np.asarray(voxel_features, np.float32)[0], (1, 2, 3, 0))
    v = np.ascontiguousarray(v).astype(np.float16)  # [z, y, x, c]
    vp = np.empty((SIZE + 1, SIZE + 1, SIZE, C), np.float16)
    vp[:SIZE, :SIZE] = v
    vp[SIZE, :SIZE] = v[SIZE - 1]
    vp[:SIZE, SIZE] = vp[:SIZE, SIZE - 1]
    vp[SIZE, SIZE] = vp[SIZE, SIZE - 1]
    vol4 = np.empty((SIZE, SIZE, SIZE, 2, 2, C), np.float16)
    for zl in range(2):
        for yl in range(2):
            vol4[:, :, :, zl, yl, :] = vp[zl : zl + SIZE, yl : yl + SIZE]
    vol4 = vol4.reshape(SIZE * SIZE * SIZE, 128)

    f8 = np.float64
    Wd = np.asarray(w_d2, f8) @ np.asarray(w_d1, f8)
    bd = np.asarray(b_d1, f8) @ np.asarray(w_d2, f8).T + np.asarray(b_d2, f8)
    Wc = np.asarray(w_c2, f8) @ np.asarray(w_c1, f8)
    bc = np.asarray(b_c1, f8) @ np.asarray(w_c2, f8).T + np.asarray(b_c2, f8)
    cw = np.asarray(conv_w, f8)[:, :, 0, :]  # [o, c', k]

    A = np.einsum("ock,cd->odk", cw, Wd)  # [o, c, k]
    M = np.moveaxis(A, 2, 0).copy()  # [k, o, c], ref order k = kx*9 + ky*3 + kz
    M[13] += Wc - A.sum(axis=2)
    bias_tot = cw.sum(axis=2) @ bd + np.asarray(conv_b, f8) + bc

    # Mbig row r = ky*288 + kz*96 + kx*32 + c maps M_{kx*9+ky*3+kz}[o, c];
    # row 864 carries the fused bias (multiplied by the constant-1 F slot)
    Mbig = np.zeros((896, C), np.float64)
    for kx in range(3):
        for ky in range(3):
            for kz in range(3):
                r0 = ky * 288 + kz * 96 + kx * 32
                Mbig[r0 : r0 + 32, :] = M[kx * 9 + ky * 3 + kz].T
    Mbig[864, :] = bias_tot
    mb_host = np.ascontiguousarray(
        Mbig.reshape(7, 128, C).transpose(1, 0, 2).reshape(128, 7 * C)
    ).astype(np.float16)
    return vol4, mb_host


def _classify(vp):
    """vp: [n, 3] f32 vertices -> (class id, gather base index), replicating
    the device's f32 arithmetic (p = fl32(v*63.5) + bias_k, floors in f32)."""
    def fl(col, bias):
        q = vp[:, col].astype(np.float32) * np.float32(SCALE_P)
        return np.floor(q + np.float32(bias)).astype(np.int64)

    x0 = fl(0, SCALE_P - DELTA_P)
    x1 = fl(0, SCALE_P)
    x2 = fl(0, SCALE_P + DELTA_P)
    y0 = fl(1, SCALE_P - DELTA_P)
    z0 = fl(2, SCALE_P - DELTA_P)
    r1 = x1 - x0
    r2 = x2 - x0
    cls = np.full(vp.shape[0], -1, np.int64)
    for i, (a, b) in enumerate(CLASS_R):
        cls[(r1 == a) & (r2 == b)] = i
    assert (cls >= 0).all(), "unexpected x-spacing class"
    base = (z0 * 128 + y0) * 128 + x0  # HBM row order, for locality sort
    return cls, base


def kernel(**inputs):
    vol4, mb_host = _host_prep(**inputs)
    vp = np.asarray(inputs["vertices"], np.float32)[0]
    n = vp.shape[0]

    # shard vertices contiguously, then sort within each core by
    # (class, gather address) for uniform code + HBM locality
    per_core = (n + N_CORES - 1) // N_CORES
    in_maps = []
    counts_ref = None
    for i in range(N_CORES):
        seg = vp[i * per_core : min((i + 1) * per_core, n)]
        cls, base = _classify(seg)
        order = np.argsort((cls << 42) + base)
        seg_sorted = seg[order]
        cls_sorted = cls[order]
        tile_counts = []
        v_parts = []
        for c in range(len(CLASS_OFFS)):
            part = seg_sorted[cls_sorted == c]
            n_t = (len(part) + 127) // 128
            if len(part) < n_t * 128:
                pad = np.repeat(part[:1], n_t * 128 - len(part), axis=0)
                part = np.concatenate([part, pad], axis=0)
            tile_counts.append(n_t)
            v_parts.append(part)
        verts_padded = np.concatenate(
            [p for p in v_parts if len(p)], axis=0
        ).astype(np.float32)
        if counts_ref is None:
            counts_ref = tuple(tile_counts)
        else:
            # all cores share one compiled program: equalize tile counts
            counts_ref = tuple(max(a, b) for a, b in zip(counts_ref, tile_counts))
        in_maps.append({"verts": verts_padded, "tile_counts": tuple(tile_counts),
                        "order": order, "seg_len": len(seg), "cls": cls})

    # pad every core's segments up to the common per-class tile counts
    for i in range(N_CORES):
        m = in_maps[i]
        tc_i = m["tile_counts"]
        v = m["verts"]
        pieces = []
        start = 0
        for c in range(len(CLASS_OFFS)):
            seg_c = v[start : start + tc_i[c] * 128]
            start += tc_i[c] * 128
            need = counts_ref[c] * 128
            if len(seg_c) < need:
                fill = seg_c[:1] if len(seg_c) else v[:1]
                seg_c = np.concatenate(
                    [seg_c, np.repeat(fill, need - len(seg_c), axis=0)], axis=0
                )
            pieces.append(seg_c)
        m["verts"] = np.ascontiguousarray(np.concatenate(pieces, axis=0))

    nc = _get_nc(counts_ref)
    run_maps = [
        {"vol": vol4, "verts": in_maps[i]["verts"], "mbig": mb_host}
        for i in range(N_CORES)
    ]
    res = run_bass_kernel_spmd(
        nc, run_maps, list(range(N_CORES)),
        trace=os.environ.get("KBENCH_TRACE", "") == "1",
    )
    globals()["LAST_RESULTS"] = res

    out = np.empty((n, C), np.float32)
    bounds = np.cumsum([0] + [c * 128 for c in counts_ref])
    for i in range(N_CORES):
        m = in_maps[i]
        seg_len = m["seg_len"]
        raw = res.results[i]["out"]
        c_of = m["cls"][m["order"]]
        vals = []
        for c in range(len(CLASS_OFFS)):
            k = int((c_of == c).sum())
            vals.append(raw[bounds[c] : bounds[c] + k])
        sorted_out = np.concatenate(vals, axis=0)
        seg_out = np.empty_like(sorted_out)
        seg_out[m["order"]] = sorted_out
        out[i * per_core : i * per_core + seg_len] = seg_out
    return out.reshape(1, n, C).astype(np.float32)


# revision 10
# speedup vs baseline: 1.5737x; 1.4284x over previous
import os
import sys

sys.path.insert(0, "/opt/trn_rl_repo")

import numpy as np
import ml_dtypes

import concourse.bass as bass
import concourse.tile as tile
import concourse.mybir as mybir
from concourse import bacc
from concourse.bass import ts
from concourse.bass_utils import run_bass_kernel_spmd

N_CORES = 8
C = 32
SIZE = 128
N_FULL = 50000

SCALE_P = 63.5  # (size-1)/2
DELTA_P = 0.0625 * 63.5  # sample spacing in pixel units = 3.96875

F32 = mybir.dt.float32
F16 = mybir.dt.float16
I32 = mybir.dt.int32

AluOp = mybir.AluOpType
ActFn = mybir.ActivationFunctionType

# x-pair offsets within the gathered 10-voxel span per class
CLASS_OFFS = [(0, 4, 8), (0, 3, 7), (0, 4, 7), (0, 3, 6)]
CLASS_R = [(4, 8), (3, 7), (4, 7), (3, 6)]

_cache = {}


def _emit_preamble(nc, cpool, verts, T):
    """Compute per-vertex fractional weights (w9a) and gather base indices
    (idxia) for ALL tiles in a handful of wide instructions."""
    vall = cpool.tile([128, T * 3], F32, tag="vall")
    nc.sync.dma_start(vall[:], verts.rearrange("(t p) a -> p t a", p=128))
    vall_v = vall[:].rearrange("p (t a) -> p t a", a=3)

    p9a = cpool.tile([128, T * 9], F32, tag="p9a")
    p9a_v = p9a[:].rearrange("p (t n) -> p t n", n=9)
    for k in range(3):
        nc.scalar.activation(
            p9a_v[:, :, k * 3 : (k + 1) * 3],
            vall_v,
            ActFn.Copy,
            bias=SCALE_P + (k - 1) * DELTA_P,
            scale=SCALE_P,
        )
    # floor + frac, robust to cast rounding mode
    ci = cpool.tile([128, T * 9], I32, tag="ci")
    nc.vector.tensor_copy(ci[:], p9a[:])
    cf = cpool.tile([128, T * 9], F32, tag="cf")
    nc.vector.tensor_copy(cf[:], ci[:])
    d9 = cpool.tile([128, T * 9], F32, tag="d9")
    nc.vector.tensor_tensor(d9[:], p9a[:], cf[:], AluOp.subtract)
    m9 = cpool.tile([128, T * 9], F32, tag="m9")
    nc.vector.tensor_scalar(m9[:], d9[:], 0.0, None, AluOp.is_lt)
    w9a = cpool.tile([128, T * 9], F32, tag="w9a")
    nc.vector.tensor_tensor(w9a[:], d9[:], m9[:], AluOp.add)
    i9a = cpool.tile([128, T * 9], F32, tag="i9a")
    nc.vector.tensor_tensor(i9a[:], cf[:], m9[:], AluOp.subtract)

    i9a_v = i9a[:].rearrange("p (t k a) -> p t k a", k=3, a=3)
    rza = cpool.tile([128, T * 3], F32, tag="rza")
    rza_v = rza[:].rearrange("p (t k) -> p t k", k=3)
    nc.vector.tensor_scalar(rza_v, i9a_v[:, :, :, 2], 16384.0, None, AluOp.mult)
    rya = cpool.tile([128, T * 3], F32, tag="rya")
    rya_v = rya[:].rearrange("p (t k) -> p t k", k=3)
    nc.vector.tensor_scalar(rya_v, i9a_v[:, :, :, 1], 128.0, None, AluOp.mult)

    # zy9a[t, kz, ky] = rza[t, kz] + rya[t, ky]
    zy9a = cpool.tile([128, T * 9], F32, tag="zy9a")
    zy9a_v = zy9a[:].rearrange("p (t kz ky) -> p t kz ky", kz=3, ky=3)
    rza_b = rza[:].rearrange("p (t kz o) -> p t kz o", kz=3, o=1).to_broadcast(
        (128, T, 3, 3)
    )
    rya_b = rya[:].rearrange("p (t o ky) -> p t o ky", o=1, ky=3).to_broadcast(
        (128, T, 3, 3)
    )
    nc.vector.tensor_tensor(zy9a_v, rza_b, rya_b, AluOp.add)

    # idx[t, j] = zy9a[t, j] + x0(t)   (x0 = i9a col 0 of each tile)
    idxfa = cpool.tile([128, T * 9], F32, tag="idxfa")
    idxfa_v = idxfa[:].rearrange("p (t n) -> p t n", n=9)
    x0_b = i9a[:].rearrange("p (t n) -> p t n", n=9)[:, :, 0:1].to_broadcast(
        (128, T, 9)
    )
    nc.vector.tensor_tensor(
        idxfa_v, zy9a[:].rearrange("p (t n) -> p t n", n=9), x0_b, AluOp.add
    )
    idxia = cpool.tile([128, T * 9], I32, tag="idxia")
    nc.vector.tensor_copy(idxia[:], idxfa[:])
    return w9a, idxia


def _emit_tile(nc, pools, tl, cls, w9a, idxia, consts):
    (gpool, dpool, wpool, xpool, zpool, fpool, pspool, opool) = pools
    mb_sb, ident_sb, vol, out = consts
    offs = CLASS_OFFS[cls]

    G = gpool.tile([128, 9, 1280], F16, tag="G")
    for j in range(9):
        nc.gpsimd.indirect_dma_start(
            out=G[:, j, :],
            out_offset=None,
            in_=vol[:, :],
            in_offset=bass.IndirectOffsetOnAxis(
                ap=idxia[:, tl * 9 + j : tl * 9 + j + 1], axis=0
            ),
        )

    Gx = G[:].rearrange("p j (x e) -> p j x e", x=10)
    # x-lerp: dxa [kx, r, e] then X[r, kx, e] = A + wx*dx
    dxa = dpool.tile([128, 3 * 9 * 128], F16, tag="dxa")
    dxa_v = dxa[:].rearrange("p (k r e) -> p k r e", k=3, r=9)
    step = offs[1] - offs[0]
    if offs[2] - offs[1] == step:
        A_t = Gx[:, :, offs[0] : offs[2] + 1 : step, :].rearrange(
            "p j k e -> p k j e"
        )
        B_t = Gx[:, :, offs[0] + 1 : offs[2] + 2 : step, :].rearrange(
            "p j k e -> p k j e"
        )
        nc.vector.tensor_tensor(dxa_v, B_t, A_t, AluOp.subtract)
    else:
        for kx in range(3):
            nc.vector.tensor_tensor(
                dxa_v[:, kx],
                Gx[:, :, offs[kx] + 1, :],
                Gx[:, :, offs[kx], :],
                AluOp.subtract,
            )

    X = xpool.tile([128, 9 * 3 * 128], F16, tag="X")
    X_v = X[:].rearrange("p (r k e) -> p r k e", r=9, k=3)
    for kx in range(3):
        wdx = wpool.tile([128, 9 * 128], F16, tag="wdx")
        wdx_v = wdx[:].rearrange("p (r e) -> p r e", r=9)
        nc.scalar.activation(
            wdx_v,
            dxa_v[:, kx],
            ActFn.Copy,
            bias=0.0,
            scale=w9a[:, tl * 9 + 3 * kx : tl * 9 + 3 * kx + 1],
        )
        nc.vector.tensor_tensor(
            X_v[:, :, kx, :], wdx_v, Gx[:, :, offs[kx], :], AluOp.add
        )

    # z-lerp: fold zl. X = [r=(kz,ky), kx, zl, ylc]
    Xz = X[:].rearrange("p (r k zl e) -> p r k zl e", r=9, k=3, zl=2)
    dza = dpool.tile([128, 27 * 64], F16, tag="dza")
    dza_v = dza[:].rearrange("p (kz ky k e) -> p kz ky k e", kz=3, ky=3, k=3)
    nc.vector.tensor_tensor(dza_v, Xz[:, :, :, 1, :].rearrange(
        "p (kz ky) k e -> p kz ky k e", kz=3
    ), Xz[:, :, :, 0, :].rearrange("p (kz ky) k e -> p kz ky k e", kz=3),
        AluOp.subtract)

    Z = zpool.tile([128, 27 * 64], F16, tag="Z")
    Z_v = Z[:].rearrange("p (ky kz k e) -> p ky kz k e", ky=3, kz=3, k=3)
    Xz4 = X[:].rearrange(
        "p (kz ky k zl e) -> p kz ky k zl e", kz=3, ky=3, k=3, zl=2
    )
    for kz in range(3):
        wdz = wpool.tile([128, 9 * 64], F16, tag="wdz")
        wdz_v = wdz[:].rearrange("p (ky k e) -> p ky k e", ky=3, k=3)
        nc.vector.tensor_scalar(
            wdz_v,
            dza_v[:, kz],
            w9a[:, tl * 9 + 3 * kz + 2 : tl * 9 + 3 * kz + 3],
            None,
            AluOp.mult,
        )
        nc.vector.tensor_tensor(
            Z_v[:, :, kz], wdz_v, Xz4[:, kz, :, :, 0, :], AluOp.add
        )

    # y-lerp: fold yl. Z = [ky, kz, kx, yl, c]
    Zy = Z[:].rearrange("p (ky a yl c) -> p ky a yl c", ky=3, a=9, yl=2)
    dya = dpool.tile([128, 27 * C], F16, tag="dya")
    dya_v = dya[:].rearrange("p (ky a c) -> p ky a c", ky=3, a=9)
    nc.vector.tensor_tensor(dya_v, Zy[:, :, :, 1, :], Zy[:, :, :, 0, :],
                            AluOp.subtract)

    F = fpool.tile([128, 896], F16, tag="F")
    F_v = F[:, 0:864].rearrange("p (ky a c) -> p ky a c", ky=3, a=9)
    for ky in range(3):
        wdy = wpool.tile([128, 9 * C], F16, tag="wdy")
        wdy_v = wdy[:].rearrange("p (a c) -> p a c", a=9)
        nc.vector.tensor_scalar(
            wdy_v,
            dya_v[:, ky],
            w9a[:, tl * 9 + 3 * ky + 1 : tl * 9 + 3 * ky + 2],
            None,
            AluOp.mult,
        )
        nc.vector.tensor_tensor(
            F_v[:, ky], wdy_v, Zy[:, ky, :, 0, :], AluOp.add
        )
    nc.vector.memset(F[:, 864:865], 1.0)
    nc.vector.memset(F[:, 865:896], 0.0)

    # transpose F via TensorE (identity matmul) — a Sync DMA_TRANSPOSE here
    # would serialize against the SWDGE gather stream (deadlock guard)
    FT = fpool.tile([128, 7, 128], F16, tag="FT")
    for t in range(7):
        ftp = pspool.tile([128, 128], F16, tag="ftp")
        nc.tensor.transpose(ftp[:], F[:, ts(t, 128)], ident_sb[:])
        nc.scalar.copy(FT[:, t, :], ftp[:])

    psum = pspool.tile([128, C], F32, tag="ps")
    for t in range(7):
        nc.tensor.matmul(
            psum[:], FT[:, t, :], mb_sb[:, ts(t, C)], start=(t == 0), stop=(t == 6)
        )
    osb = opool.tile([128, C], F32, tag="osb")
    nc.scalar.copy(osb[:], psum[:])
    nc.sync.dma_start(out[ts(tl, 128), :], osb[:])


def _build(tile_counts):
    """tile_counts: per-class 128-vertex tile counts."""
    T = sum(tile_counts)
    nv = T * 128
    nc = bacc.Bacc("TRN2", target_bir_lowering=False, debug=False)

    vol = nc.dram_tensor(
        "vol", [SIZE * SIZE * SIZE, 128], F16, kind="ExternalInput"
    ).ap()  # [z y x] rows of [zl yl c] = 128 els
    verts = nc.dram_tensor("verts", [nv, 3], F32, kind="ExternalInput").ap()
    mbig = nc.dram_tensor("mbig", [128, 7 * C], F16, kind="ExternalInput").ap()
    ident = nc.dram_tensor("ident", [128, 128], F16, kind="ExternalInput").ap()
    out = nc.dram_tensor("out", [nv, C], F32, kind="ExternalOutput").ap()

    with tile.TileContext(nc) as tc:
        with (
            tc.tile_pool(name="const", bufs=1) as cpool,
            tc.tile_pool(name="gather", bufs=4) as gpool,
            tc.tile_pool(name="dd", bufs=3) as dpool,
            tc.tile_pool(name="wd", bufs=3) as wpool,
            tc.tile_pool(name="xl", bufs=2) as xpool,
            tc.tile_pool(name="zl", bufs=2) as zpool,
            tc.tile_pool(name="fl", bufs=3) as fpool,
            tc.tile_pool(name="psum", bufs=4, space="PSUM") as pspool,
            tc.tile_pool(name="outp", bufs=3) as opool,
        ):
            mb_sb = cpool.tile([128, 7 * C], F16, tag="mb")
            nc.sync.dma_start(mb_sb[:], mbig[:])
            ident_sb = cpool.tile([128, 128], F16, tag="ident")
            nc.sync.dma_start(ident_sb[:], ident[:])
            w9a, idxia = _emit_preamble(nc, cpool, verts, T)
            pools = (gpool, dpool, wpool, xpool, zpool, fpool, pspool, opool)
            consts = (mb_sb, ident_sb, vol, out)

            tl = 0
            for cls, n_t in enumerate(tile_counts):
                for _ in range(n_t):
                    _emit_tile(nc, pools, tl, cls, w9a, idxia, consts)
                    tl += 1

    nc.compile()
    return nc


def _get_nc(tile_counts):
    key = tuple(tile_counts)
    if key not in _cache:
        _cache[key] = _build(key)
    return _cache[key]


def _host_prep(voxel_features, vertices, w_d1, b_d1, w_d2, b_d2, w_c1, b_c1, w_c2,
               b_c2, conv_w, conv_b):
    # volume -> [z, y, x, zl, yl, c] fp16 (x4 redundant corner-pair layout)
    v = np.transpose(np.asarray(voxel_features, np.float32)[0], (1, 2, 3, 0))
    v = np.ascontiguousarray(v).astype(np.float16)  # [z, y, x, c]
    vp = np.empty((SIZE + 1, SIZE + 1, SIZE, C), np.float16)
    vp[:SIZE, :SIZE] = v
    vp[SIZE, :SIZE] = v[SIZE - 1]
    vp[:SIZE, SIZE] = vp[:SIZE, SIZE - 1]
    vp[SIZE, SIZE] = vp[SIZE, SIZE - 1]
    vol4 = np.empty((SIZE, SIZE, SIZE, 2, 2, C), np.float16)
    for zl in range(2):
        for yl in range(2):
            vol4[:, :, :, zl, yl, :] = vp[zl : zl + SIZE, yl : yl + SIZE]
    vol4 = vol4.reshape(SIZE * SIZE * SIZE, 128)

    f8 = np.float64
    Wd = np.asarray(w_d2, f8) @ np.asarray(w_d1, f8)
    bd = np.asarray(b_d1, f8) @ np.asarray(w_d2, f8).T + np.asarray(b_d2, f8)
    Wc = np.asarray(w_c2, f8) @ np.asarray(w_c1, f8)
    bc = np.asarray(b_c1, f8) @ np.asarray(w_c2, f8).T + np.asarray(b_c2, f8)
    cw = np.asarray(conv_w, f8)[:, :, 0, :]  # [o, c', k]

    A = np.einsum("ock,cd->odk", cw, Wd)  # [o, c, k]
    M = np.moveaxis(A, 2, 0).copy()  # [k, o, c], ref order k = kx*9 + ky*3 + kz
    M[13] += Wc - A.sum(axis=2)
    bias_tot = cw.sum(axis=2) @ bd + np.asarray(conv_b, f8) + bc

    # Mbig row r = ky*288 + kz*96 + kx*32 + c maps M_{kx*9+ky*3+kz}[o, c];
    # row 864 carries the fused bias (multiplied by the constant-1 F slot)
    Mbig = np.zeros((896, C), np.float64)
    for kx in range(3):
        for ky in range(3):
            for kz in range(3):
                r0 = ky * 288 + kz * 96 + kx * 32
                Mbig[r0 : r0 + 32, :] = M[kx * 9 + ky * 3 + kz].T
    Mbig[864, :] = bias_tot
    mb_host = np.ascontiguousarray(
        Mbig.reshape(7, 128, C).transpose(1, 0, 2).reshape(128, 7 * C)
    ).astype(np.float16)
    return vol4, mb_host


def _classify(vp):
    """vp: [n, 3] f32 vertices -> (class id, gather base index), replicating
    the device's f32 arithmetic (p = fl32(v*63.5) + bias_k, floors in f32)."""
    def fl(col, bias):
        q = vp[:, col].astype(np.float32) * np.float32(SCALE_P)
        return np.floor(q + np.float32(bias)).astype(np.int64)

    x0 = fl(0, SCALE_P - DELTA_P)
    x1 = fl(0, SCALE_P)
    x2 = fl(0, SCALE_P + DELTA_P)
    y0 = fl(1, SCALE_P - DELTA_P)
    z0 = fl(2, SCALE_P - DELTA_P)
    r1 = x1 - x0
    r2 = x2 - x0
    cls = np.full(vp.shape[0], -1, np.int64)
    for i, (a, b) in enumerate(CLASS_R):
        cls[(r1 == a) & (r2 == b)] = i
    assert (cls >= 0).all(), "unexpected x-spacing class"
    base = (z0 * 128 + y0) * 128 + x0  # HBM row order, for locality sort
    return cls, base


def kernel(**inputs):
    vol4, mb_host = _host_prep(**inputs)
    vp = np.asarray(inputs["vertices"], np.float32)[0]
    n = vp.shape[0]

    # shard vertices contiguously, then sort within each core by
    # (class, gather address) for uniform code + HBM locality
    per_core = (n + N_CORES - 1) // N_CORES
    in_maps = []
    counts_ref = None
    for i in range(N_CORES):
        seg = vp[i * per_core : min((i + 1) * per_core, n)]
        cls, base = _classify(seg)
        order = np.argsort((cls << 42) + base)
        seg_sorted = seg[order]
        cls_sorted = cls[order]
        tile_counts = []
        v_parts = []
        for c in range(len(CLASS_OFFS)):
            part = seg_sorted[cls_sorted == c]
            n_t = (len(part) + 127) // 128
            if len(part) < n_t * 128:
                pad = np.repeat(part[:1], n_t * 128 - len(part), axis=0)
                part = np.concatenate([part, pad], axis=0)
            tile_counts.append(n_t)
            v_parts.append(part)
        verts_padded = np.concatenate(
            [p for p in v_parts if len(p)], axis=0
        ).astype(np.float32)
        if counts_ref is None:
            counts_ref = tuple(tile_counts)
        else:
            # all cores share one compiled program: equalize tile counts
            counts_ref = tuple(max(a, b) for a, b in zip(counts_ref, tile_counts))
        in_maps.append({"verts": verts_padded, "tile_counts": tuple(tile_counts),
                        "order": order, "seg_len": len(seg), "cls": cls})

    # pad every core's segments up to the common per-class tile counts
    for i in range(N_CORES):
        m = in_maps[i]
        tc_i = m["tile_counts"]
        v = m["verts"]
        pieces = []
        start = 0
        for c in range(len(CLASS_OFFS)):
            seg_c = v[start : start + tc_i[c] * 128]
            start += tc_i[c] * 128
            need = counts_ref[c] * 128
            if len(seg_c) < need:
                fill = seg_c[:1] if len(seg_c) else v[:1]
                seg_c = np.concatenate(
                    [seg_c, np.repeat(fill, need - len(seg_c), axis=0)], axis=0
                )
            pieces.append(seg_c)
        m["verts"] = np.ascontiguousarray(np.concatenate(pieces, axis=0))

    nc = _get_nc(counts_ref)
    ident_host = np.eye(128, dtype=np.float16)
    run_maps = [
        {"vol": vol4, "verts": in_maps[i]["verts"], "mbig": mb_host,
         "ident": ident_host}
        for i in range(N_CORES)
    ]
    res = run_bass_kernel_spmd(
        nc, run_maps, list(range(N_CORES)),
        trace=os.environ.get("KBENCH_TRACE", "") == "1",
    )
    globals()["LAST_RESULTS"] = res

    out = np.empty((n, C), np.float32)
    bounds = np.cumsum([0] + [c * 128 for c in counts_ref])
    for i in range(N_CORES):
        m = in_maps[i]
        seg_len = m["seg_len"]
        raw = res.results[i]["out"]
        c_of = m["cls"][m["order"]]
        vals = []
        for c in range(len(CLASS_OFFS)):
            k = int((c_of == c).sum())
            vals.append(raw[bounds[c] : bounds[c] + k])
        sorted_out = np.concatenate(vals, axis=0)
        seg_out = np.empty_like(sorted_out)
        seg_out[m["order"]] = sorted_out
        out[i * per_core : i * per_core + seg_len] = seg_out
    return out.reshape(1, n, C).astype(np.float32)
